# revision 36
# baseline (speedup 1.0000x reference)
"""Trainium2 Bass kernel for nn_PixelWiseAdpNet.

Sharding: the (batch=4) x (patch-row-half=2) grid -> 8 cores; each core owns
one batch's 4x8 block of patches (32 patches, 16384 points) and runs:
  phase inX : inX = coord_em + w_cd @ coord_data^T            (PE + DVE)
  phase A   : per-patch hyper-MLP params = w_feat @ F + b_feat
              (w_feat streamed as the stationary operand, c on partitions
               so param tiles are directly consumable as W^T K-tiles;
              stream order: [b1 b2 b3 pad | W1' | pad | W2 | W3] so biases
              and early layers unblock MLP work while the stream continues)
  phase MLP : 3-layer per-patch MLP with LeakyReLU            (PE + ACT)
The W2 region (72% of the stream bytes) is stored in HBM as fp8 e3m4
scaled by 256 (exact power of two), consumed by the phase-A matmul as the
fp8 stationary against a 1/256-scaled copy of the patch features: halves
the dominant DMA stream (46.5MB -> 29.9MB per core) at +9e-3 absmax err.
The stream is padded so the fp8 region starts exactly at slab 5 and the
W2h0/W2h1/W3 slab-group boundaries of the interleave are unchanged.
All matmul inputs are bf16/fp8 (fp32 PSUM accumulation); pointwise fp32.
MLP emission is interleaved with the w_feat stream slabs so L1/L2 run in
the stream's DMA shadow; L3 waits for the W3 region (end of stream).
"""

import numpy as np
import ml_dtypes

import concourse.mybir as mybir
import concourse.tile as tile
from concourse import bacc
from concourse.bass_utils import run_bass_kernel_spmd
from concourse.masks import make_identity

BF16 = ml_dtypes.bfloat16
E3 = ml_dtypes.float8_e3m4

B, IN_CH, OUT_CH, FEAT_CH = 4, 32, 64, 256
AH = AW = 8
OUT_H = OUT_W = 64
S = 8
NEG = 0.01
C_TOTAL = 90688
C_PAD = 92160            # padded so W2 starts at tile 80 (slab 5)
N_CORES = 8
N_TILES = C_PAD // 128   # 720
NQ = 32                  # patches (pairs) per core

# tile indices in the REORDERED + padded c stream
#   [b1:0-1][b2:2-3][b3+pad:4][W1':5-68][pad:69-79][W2:80-591][W3:592-719]
T_W1 = 5
T_W2 = 80
T_W3 = 592
C_W2_0 = T_W2 * 128      # 10240
C_W3_0 = T_W3 * 128      # 75776
W2_SCALE = 256.0

SLAB_C = 2048            # c columns per w_feat DMA slab (16 tiles)
F32 = mybir.dt.float32
BF = mybir.dt.bfloat16
F8 = mybir.dt.float8e3

_CACHE = {}


def _build(variant="all"):
    nreps = 1
    if variant.startswith("rep"):
        rep, _, rest = variant.partition(":")
        nreps = int(rep[3:])
        variant = rest or "all"
    nc = bacc.Bacc("TRN2", target_bir_lowering=False, debug=False,
                   num_devices=N_CORES)

    whead_d = nc.dram_tensor("whead", [2, 128, C_W2_0], BF,
                             kind="ExternalInput")
    w2_d = nc.dram_tensor("w2", [2, 128, C_W3_0 - C_W2_0], F8,
                          kind="ExternalInput")
    w3_d = nc.dram_tensor("w3", [2, 128, C_PAD - C_W3_0], BF,
                          kind="ExternalInput")
    bfeat_d = nc.dram_tensor("bfeat", [128, N_TILES], F32, kind="ExternalInput")
    mlpfT_d = nc.dram_tensor("mlpfT", [128, 2, NQ], BF, kind="ExternalInput")
    mlpfs_d = nc.dram_tensor("mlpfs", [128, 2, NQ], BF, kind="ExternalInput")
    em_d = nc.dram_tensor("em", [IN_CH, S, 32, 64], F32, kind="ExternalInput")
    cdT_d = nc.dram_tensor("cdT", [OUT_CH + 1, S, 32, 64], BF, kind="ExternalInput")
    wcdT_d = nc.dram_tensor("wcdT", [OUT_CH + 1, IN_CH], BF, kind="ExternalInput")
    out_d = nc.dram_tensor("out", [OUT_CH, 8, S, 8, 32], F32,
                           kind="ExternalOutput")

    with tile.TileContext(nc) as tc:
        with (
            tc.tile_pool(name="const", bufs=1) as const_pool,
            tc.tile_pool(name="wstream", bufs=4) as wpool,
            tc.tile_pool(name="params", bufs=1) as ppool,
            tc.tile_pool(name="coords", bufs=2) as cpool,
            tc.tile_pool(name="acts", bufs=3) as apool,
            tc.tile_pool(name="outrow", bufs=2) as opool,
            tc.tile_pool(name="psA", bufs=2, space="PSUM") as psA,
            tc.tile_pool(name="psM", bufs=2, space="PSUM") as psM,
            tc.tile_pool(name="psY", bufs=4, space="PSUM") as psY,
        ):
            for _rep in range(nreps):
                # ---- constants ----
                ident = const_pool.tile([128, 128], BF, name="ident")
                make_identity(nc, ident[:])
                mlpfT = const_pool.tile([128, 2, NQ], BF, name="mlpfT")
                nc.sync.dma_start(mlpfT[:], mlpfT_d[:])
                mlpfs = const_pool.tile([128, 2, NQ], BF, name="mlpfs")
                nc.sync.dma_start(mlpfs[:], mlpfs_d[:])
                wcdT = const_pool.tile([OUT_CH + 1, IN_CH], BF, name="wcdT")
                nc.sync.dma_start(wcdT[:], wcdT_d[:])
                bfeat = const_pool.tile([128, N_TILES], F32, name="bfeat")
                nc.sync.dma_start(bfeat[:], bfeat_d[:])

                # params split per region so consumers only dep their region
                REG = [(0, T_W2), (T_W2, T_W3), (T_W3, N_TILES)]
                pregs = [
                    ppool.tile([128, T_W2, NQ], BF, name="pW1"),
                    ppool.tile([128, T_W3 - T_W2, NQ], BF, name="pW2"),
                    ppool.tile([128, N_TILES - T_W3, NQ], BF, name="pW3"),
                ]
                bias_sb = ppool.tile([128, 5, NQ], F32, name="bias_sb")
                bias_map = {0: 0, 1: 1, 2: 2, 3: 3, 4: 4}

                def preg(t):
                    for (lo, hi), pt in zip(REG, pregs):
                        if lo <= t < hi:
                            return pt, t - lo
                    raise AssertionError(t)

                if variant == "noA":
                    for pt in pregs:
                        nc.vector.memset(pt[:], 0.0)
                    nc.vector.memset(bias_sb[:], 0.0)

                n_slabs = 0 if variant == "noA" else C_PAD // SLAB_C

                def emit_slab(sl):
                    c0 = sl * SLAB_C
                    cw = min(SLAB_C, C_PAD - c0)
                    if sl == 4:
                        cw = 640        # skip the zero pad (tiles 69-79)
                    if cw <= 0 or sl >= n_slabs:
                        return
                    t0 = c0 // 128
                    ntile_sl = cw // 128
                    if sl < 5:
                        dram, base, dt, rhs = whead_d, 0, BF, mlpfT
                    elif sl < 37:
                        dram, base, dt, rhs = w2_d, C_W2_0, F8, mlpfs
                    else:
                        dram, base, dt, rhs = w3_d, C_W3_0, BF, mlpfT
                    wbuf = wpool.tile([128, 2, SLAB_C], dt, name="wbuf")
                    for k in range(2):
                        nc.sync.dma_start(wbuf[:, k, :cw],
                                          dram[k, :, c0 - base:c0 - base + cw])
                    ps = psA.tile([128, SLAB_C // 128, NQ], F32, name="ps")
                    for u in range(ntile_sl):
                        for k in range(2):
                            nc.tensor.matmul(
                                ps[:, u, :],
                                wbuf[:, k, u * 128:(u + 1) * 128],
                                rhs[:, k, :],
                                start=(k == 0), stop=(k == 1))
                    u = 0
                    while u < ntile_sl:
                        t = t0 + u
                        pt, lt = preg(t)
                        seg = min(ntile_sl - u,
                                  next(hi for (lo, hi) in REG if lo <= t < hi) - t)
                        nc.vector.tensor_tensor(
                            out=pt[:, lt:lt + seg, :],
                            in0=ps[:, u:u + seg, :],
                            in1=bfeat[:, t:t + seg].unsqueeze(2).broadcast_to(
                                (128, seg, NQ)),
                            op=mybir.AluOpType.add)
                        u += seg
                    for t in bias_map:
                        if t0 <= t < t0 + ntile_sl:
                            nc.vector.tensor_scalar_add(
                                bias_sb[:, bias_map[t], :], ps[:, t - t0, :],
                                bfeat[:, t:t + 1])

                mlp_on = variant != "nomlp"
                x1s, x2h0s, x2h1s = {}, {}, {}

                with tc.tile_pool(name="early", bufs=1) as epool:
                    # ---- phase inX: x0[i, (row, s, ph, ow)] ----
                    # chunks interleave with the first slabs so the static
                    # DVE order alternates x0-adds with param drains
                    x0 = epool.tile([IN_CH, 4, S, 8, 64], BF, name="x0")

                    def emit_inx(r, s):
                        em_t = epool.tile([IN_CH, 8, 64], F32,
                                          name="em_t", bufs=2)
                        nc.scalar.dma_start(em_t[:],
                                          em_d[:, s, 8 * r:8 * r + 8, :])
                        cd_t = epool.tile([OUT_CH + 1, 8, 64], BF,
                                          name="cd_t", bufs=2)
                        nc.scalar.dma_start(cd_t[:],
                                          cdT_d[:, s, 8 * r:8 * r + 8, :])
                        cd_ps = psM.tile([IN_CH, 8, 64], F32, name="cd_ps", tag="cd", bufs=1)
                        nc.tensor.matmul(cd_ps[:], wcdT[:], cd_t[:],
                                         start=True, stop=True)
                        nc.vector.tensor_tensor(
                            out=x0[:, r, s, :, :],
                            in0=cd_ps[:], in1=em_t[:],
                            op=mybir.AluOpType.add)

                    chunks = [(r, s) for r in range(4) for s in range(S)]
                    for sl in range(0, 5):       # biases + W1' + pad
                        emit_slab(sl)
                        for (r, s) in chunks[sl * 7:(sl + 1) * 7]:
                            emit_inx(r, s)
                    for (r, s) in chunks[35:]:
                        emit_inx(r, s)

                    if mlp_on:
                        # W1 fixup + L1 for ALL pairs (x1 stays live to L2)
                        for q in range(NQ):
                            r, wp = q // 8, q % 8
                            w1T = epool.tile([IN_CH, 256], BF, name="w1T",
                                             bufs=6)
                            tp = psM.tile([IN_CH, 256], BF, name="tp", tag="tp", bufs=1)
                            for h in range(2):
                                nc.tensor.transpose(
                                    tp[:, 128 * h:128 * h + 128],
                                    pregs[0][:, T_W1 + h:T_W1 + h + 63:2, q],
                                    ident[:])
                            nc.vector.tensor_copy(w1T[:], tp[:])
                            xq = x0[:, r, :, :, 8 * wp:8 * wp + 8]
                            x1 = apool.tile([128, 2, 512], BF, name="x1",
                                            bufs=NQ)
                            for h in range(2):
                                y1 = psY.tile([128, 512], F32, name="y1",
                                              tag="y")
                                nc.tensor.matmul(
                                    y1[:], w1T[:, 128 * h:128 * h + 128],
                                    xq, start=True, stop=True)
                                nc.scalar.activation(
                                    x1[:, h, :], y1[:],
                                    mybir.ActivationFunctionType.Lrelu,
                                    bias=bias_sb[:, h, q:q + 1], scale=1.0,
                                    alpha=NEG)
                            x1s[q] = x1

                with tc.tile_pool(name="late", bufs=1) as lpool:
                    for sl in range(5, 21):      # W2 h0-reachable part
                        emit_slab(sl)
                    if mlp_on:
                        # L2 h=0 for all pairs (W2 local tiles 0..255 ready)
                        for q in range(NQ):
                            x2 = lpool.tile([128, 512], BF, name="x2h0",
                                            bufs=NQ)
                            y2 = psY.tile([128, 512], F32, name="y2", tag="y")
                            for k in range(2):
                                nc.tensor.matmul(
                                    y2[:], pregs[1][:, k:k + 255:2, q],
                                    x1s[q][:, k, :], start=(k == 0),
                                    stop=(k == 1))
                            nc.scalar.activation(
                                x2[:], y2[:],
                                mybir.ActivationFunctionType.Lrelu,
                                bias=bias_sb[:, 2, q:q + 1], scale=1.0,
                                alpha=NEG)
                            x2h0s[q] = x2

                    H1E = 12  # x2h1 bufs; L2h1 beyond this interleaves L3

                    def emit_l2h1(q):
                        x2 = lpool.tile([128, 512], BF, name="x2h1", bufs=H1E)
                        y2 = psY.tile([128, 512], F32, name="y2", tag="y")
                        for k in range(2):
                            t0 = 256 + k
                            nc.tensor.matmul(
                                y2[:], pregs[1][:, t0:t0 + 255:2, q],
                                x1s[q][:, k, :], start=(k == 0),
                                stop=(k == 1))
                        nc.scalar.activation(
                            x2[:], y2[:],
                            mybir.ActivationFunctionType.Lrelu,
                            bias=bias_sb[:, 3, q:q + 1], scale=1.0,
                            alpha=NEG)
                        x2h1s[q] = x2

                    out_state = {}

                    def emit_l3(q):
                        r, wp = q // 8, q % 8
                        hr = 2 * r + wp // 4
                        if wp % 4 == 0:
                            out_state[hr] = lpool.tile(
                                [OUT_CH, S, 8, 32], F32,
                                name="out_row", bufs=2)
                        y3 = psY.tile([OUT_CH, 512], F32, name="y3", tag="y")
                        x2h = [x2h0s[q], x2h1s[q]]
                        for k in range(2):
                            nc.tensor.matmul(
                                y3[:], pregs[2][:, k:k + 127:2, q],
                                x2h[k][:], start=(k == 0), stop=(k == 1))
                        wo = (wp % 4) * 8
                        nc.vector.tensor_scalar_add(
                            out_state[hr][:, :, :, wo:wo + 8], y3[:],
                            bias_sb[:OUT_CH, 4, q:q + 1])
                        if wp % 4 == 3:
                            nc.sync.dma_start(out_d[:, hr, :, :, :],
                                              out_state[hr][:])

                    for sl in range(21, 37):     # rest of W2
                        emit_slab(sl)
                    if mlp_on:
                        for q in range(H1E):
                            emit_l2h1(q)

                    for sl in range(37, n_slabs):  # W3
                        emit_slab(sl)

                    if variant == "nomlp":
                        for hr in range(8):
                            out_row = lpool.tile([OUT_CH, S, 8, 32], F32,
                                                 name="out_rowM", bufs=2)
                            nc.vector.memset(out_row[:], 0.0)
                            nc.sync.dma_start(out_d[:, hr, :, :, :],
                                              out_row[:])
                    if mlp_on:
                        # interleave: L3(i) frees the x2h1 slot L2h1(i+H1E)
                        # needs; emit the releaser first (FIFO tag queues)
                        for q in range(H1E, NQ):
                            emit_l3(q - H1E)
                            emit_l2h1(q)
                        for q in range(NQ - H1E, NQ):
                            emit_l3(q)

    nc.compile()
    return nc


def _host_prep(MLP_feature, coord_em, coord_data, w_cd, b_cd, w_feat, b_feat):
    # build the reordered + padded stream:
    # [b1 256][b2 256][b3 64][pad][W1' 8192 (c'=i*256+o)][pad][W2][W3]
    j = np.arange(8192)
    w1_perm = (j % 256) * 32 + (j // 256)           # orig c of W1' position j
    w_feat_r = np.zeros((C_PAD, FEAT_CH), np.float32)
    b_feat_r = np.zeros(C_PAD, np.float32)

    def put(dst0, src_idx):
        w_feat_r[dst0:dst0 + len(src_idx)] = w_feat[src_idx]
        b_feat_r[dst0:dst0 + len(src_idx)] = b_feat[src_idx]

    put(0, np.arange(8192, 8448))          # b1
    put(256, np.arange(73984, 74240))      # b2
    put(512, np.arange(90624, 90688))      # b3
    put(640, w1_perm)                      # W1'
    put(C_W2_0, np.arange(8448, 73984))    # W2 (fp8 region)
    put(C_W3_0, np.arange(74240, 90624))   # W3

    def wt(lo, hi, dt, scale=1.0):
        return np.ascontiguousarray(
            (w_feat_r[lo:hi].T * scale).reshape(2, 128, hi - lo)).astype(dt)

    whead = wt(0, C_W2_0, BF16)
    w2q = wt(C_W2_0, C_W3_0, E3, W2_SCALE)
    w3q = wt(C_W3_0, C_PAD, BF16)
    bfeat_t = np.ascontiguousarray(b_feat_r.reshape(N_TILES, 128).T)

    wcdT = np.concatenate([w_cd.T, b_cd[None, :]], 0).astype(BF16)

    in_maps = []
    for core in range(N_CORES):
        b, hh = core // 2, core % 2
        mf = MLP_feature[b, :, 4 * hh:4 * hh + 4, :].reshape(2, 128, NQ)
        mlpfT = np.ascontiguousarray(mf.transpose(1, 0, 2)).astype(BF16)
        mlpfs = np.ascontiguousarray(
            mf.transpose(1, 0, 2) / W2_SCALE).astype(BF16)
        em = np.ascontiguousarray(
            coord_em[b].reshape(IN_CH, S, OUT_H, OUT_W)[:, :, 32 * hh:32 * hh + 32, :])
        cd = coord_data[b].reshape(S, OUT_H, OUT_W, OUT_CH)[:, 32 * hh:32 * hh + 32]
        cdT = np.empty((OUT_CH + 1, S, 32, OUT_W), BF16)
        cdT[:OUT_CH] = cd.transpose(3, 0, 1, 2).astype(BF16)
        cdT[OUT_CH] = 1.0
        in_maps.append({
            "whead": whead, "w2": w2q, "w3": w3q, "bfeat": bfeat_t,
            "mlpfT": mlpfT, "mlpfs": mlpfs,
            "em": em, "cdT": cdT, "wcdT": wcdT,
        })
    return in_maps


def kernel(**inputs):
    inputs = {k: np.asarray(v) for k, v in inputs.items()}
    if "nc" not in _CACHE:
        _CACHE["nc"] = _build()
    nc = _CACHE["nc"]
    in_maps = _host_prep(**inputs)
    res = run_bass_kernel_spmd(nc, in_maps, core_ids=list(range(N_CORES)))
    out = np.empty((B, OUT_CH, S, OUT_H, OUT_W), np.float32)
    for core in range(N_CORES):
        b, hh = core // 2, core % 2
        o = res.results[core]["out"]                     # [64, 8, S, 8, 32]
        o = o.reshape(OUT_CH, 4, 2, S, 8, 32)            # (oc, r, h2, s, ph, w2)
        out[b, :, :, 32 * hh:32 * hh + 32, :] = (
            o.transpose(0, 3, 1, 4, 2, 5).reshape(OUT_CH, S, 32, OUT_W))
    return out
